# revision 6
# baseline (speedup 1.0000x reference)
"""DIEN GRU (dynamic_rnn with GRUCell + sequence_length masking) on 8 TRN2 cores.

Strategy (data-parallel over batch):
 - B=1024 batch rows are sorted by seq_len (desc) and dealt round-robin to the
   8 cores, so every core gets a stratified shard of 128 rows with an almost
   identical seq_len profile. Within a core, rows are sorted desc, so at step t
   only a prefix of k_t columns is still alive; ops are sized to that prefix.
 - Layout on device: channels on partitions, batch on the free dim.
   Host pre-transposes x to xT[d, t*128+b] and inverse-transposes the output.
 - GRU cell per step (PSUM bank regions r|u|c):
     pre_r = Wx_r@x + b_r + Wh_r@h        (h fed as q - p via two matmuls)
     pre_v = -(Wx_u@x + b_u + Wh_u@h)     (negated weights -> sigmoid gives
                                           v = 1-u directly)
     r = sigmoid(pre_r); v = sigmoid(pre_v)
     pre_c = Wc_x@x + Wc_h@(r*h); c = tanh(pre_c)
     q = v*c ; p = (v-1)*h ; h' = q - p   ( == u*h + (1-u)*c exactly )
 - Outputs y_t = h_{t+1} * mask_t; the mask multiply also zeroes the columns
   whose state is garbage (t >= seq_len), so no state-hold is needed.
 - Matmuls run in fp16 (inputs rounded to fp16); the recurrent state h is kept
   in fp32 (q/p are computed twice: fp16 copies feed the PE, fp32 copies form
   h). PSUM accumulation is fp32.
"""

import os
import numpy as np

B, T, D, H = 1024, 200, 128, 128
N_CORES = 8
BL = B // N_CORES  # 128 rows per core
CH = 32            # time steps per DMA chunk
KROUND = 8         # round alive-prefix up to multiple of this
MASK_GROUP = 4     # steps per mask-multiply group

F16 = "float16"

_compiled_cache: dict = {}


def _round_up(x, m):
    return ((x + m - 1) // m) * m


def _build_program(k_common, t_eff):
    """Build + compile the bass program. k_common: list of T ints."""
    from contextlib import ExitStack

    import concourse.tile as tile
    from concourse import bacc, mybir

    f32 = mybir.dt.float32
    f16 = mybir.dt.float16

    nc = bacc.Bacc("TRN2", target_bir_lowering=False, debug=False,
                   num_devices=N_CORES)

    xT_d = nc.dram_tensor("xT16", [D, T * BL], f16, kind="ExternalInput").ap()
    mrow_d = nc.dram_tensor("maskrow", [1, T * BL], f16, kind="ExternalInput").ap()
    wgx_d = nc.dram_tensor("wgx", [D, 2 * H], f16, kind="ExternalInput").ap()
    wghq_d = nc.dram_tensor("wghq", [H, 2 * H], f16, kind="ExternalInput").ap()
    wghp_d = nc.dram_tensor("wghp", [H, 2 * H], f16, kind="ExternalInput").ap()
    wcx_d = nc.dram_tensor("wcx", [D, H], f16, kind="ExternalInput").ap()
    wch_d = nc.dram_tensor("wch", [H, H], f16, kind="ExternalInput").ap()
    br_d = nc.dram_tensor("br", [1, H], f16, kind="ExternalInput").ap()
    bu_d = nc.dram_tensor("bu", [1, H], f16, kind="ExternalInput").ap()
    bc_d = nc.dram_tensor("bc", [1, H], f16, kind="ExternalInput").ap()
    yT_d = nc.dram_tensor("yT", [H, T * BL], f32, kind="ExternalOutput").ap()

    n_chunks = (T + CH - 1) // CH

    with tile.TileContext(nc) as tc:
        with ExitStack() as ctx:
            wpool = ctx.enter_context(tc.tile_pool(name="w", bufs=1))
            xpool = ctx.enter_context(tc.tile_pool(name="x", bufs=3))
            ypool = ctx.enter_context(tc.tile_pool(name="y", bufs=3))
            pp = ctx.enter_context(tc.tile_pool(name="gbank", bufs=3, space="PSUM"))
            cpp = ctx.enter_context(tc.tile_pool(name="cbank", bufs=3, space="PSUM"))
            mbp = ctx.enter_context(tc.tile_pool(name="mb", bufs=2, space="PSUM"))
            rp = ctx.enter_context(tc.tile_pool(name="r", bufs=3))
            vp = ctx.enter_context(tc.tile_pool(name="v", bufs=3))
            cp = ctx.enter_context(tc.tile_pool(name="c", bufs=3))
            rhp = ctx.enter_context(tc.tile_pool(name="rh", bufs=3))
            q16p = ctx.enter_context(tc.tile_pool(name="q16", bufs=3))
            q32p = ctx.enter_context(tc.tile_pool(name="q32", bufs=3))
            p16p = ctx.enter_context(tc.tile_pool(name="p16", bufs=3))
            p32p = ctx.enter_context(tc.tile_pool(name="p32", bufs=3))

            # weights / constants, loaded once
            wgx = wpool.tile([D, 2 * H], f16)
            nc.sync.dma_start(wgx[:], wgx_d[:])
            wghq = wpool.tile([H, 2 * H], f16)
            nc.sync.dma_start(wghq[:], wghq_d[:])
            wghp = wpool.tile([H, 2 * H], f16)
            nc.sync.dma_start(wghp[:], wghp_d[:])
            wcx = wpool.tile([D, H], f16)
            nc.sync.dma_start(wcx[:], wcx_d[:])
            wch = wpool.tile([H, H], f16)
            nc.sync.dma_start(wch[:], wch_d[:])
            br = wpool.tile([1, H], f16)
            nc.sync.dma_start(br[:], br_d[:])
            bu = wpool.tile([1, H], f16)
            nc.sync.dma_start(bu[:], bu_d[:])
            bc = wpool.tile([1, H], f16)
            nc.sync.dma_start(bc[:], bc_d[:])
            mrow = wpool.tile([1, T * BL], f16)
            nc.sync.dma_start(mrow[:], mrow_d[:])
            ones = wpool.tile([1, BL], f16)
            nc.gpsimd.memset(ones[:], 1.0)

            yw_prev = None
            q16_prev = p16_prev = None

            for ci in range(n_chunks):
                t0c = ci * CH
                nsteps = min(CH, t_eff - t0c)  # steps with compute
                nslots = min(CH, T - t0c)

                yw = ypool.tile([H, CH * BL], f32)
                nc.gpsimd.memset(yw[:, : nslots * BL], 0.0)

                if nsteps > 0:
                    xc = xpool.tile([D, CH * BL], f16)
                    half = (nsteps * BL) // 2
                    if half > 0:
                        nc.sync.dma_start(xc[:, :half],
                                          xT_d[:, t0c * BL: t0c * BL + half])
                        nc.sync.dma_start(
                            xc[:, half: nsteps * BL],
                            xT_d[:, t0c * BL + half: (t0c + nsteps) * BL])
                    else:
                        nc.sync.dma_start(xc[:, : nsteps * BL],
                                          xT_d[:, t0c * BL: (t0c + nsteps) * BL])

                group_start = 0
                for j in range(nsteps):
                    t = t0c + j
                    k = k_common[t]
                    hs = j * BL

                    # One PSUM accumulation group per bank: start=True on the
                    # first matmul, stop=True on the chronologically last one;
                    # reads only after stop. Gates (r|u) and cand (c) live in
                    # separate banks because the cand accumulation (rh part)
                    # happens after the gates bank is already being read.
                    gbank = pp.tile([H, 2 * BL], f32)
                    cbank = cpp.tile([H, BL], f32)
                    xs = xc[:, hs: hs + k]
                    # x parts
                    nc.tensor.matmul(gbank[:, 0:k], wgx[:, 0:H], xs,
                                     start=True, stop=False)
                    nc.tensor.matmul(gbank[:, BL: BL + k], wgx[:, H: 2 * H], xs,
                                     start=False, stop=False)
                    nc.tensor.matmul(cbank[:, 0:k], wcx[:], xs,
                                     start=True, stop=False)
                    # biases
                    nc.tensor.matmul(gbank[:, 0:k], br[:], ones[:, 0:k],
                                     start=False, stop=False)
                    nc.tensor.matmul(gbank[:, BL: BL + k], bu[:], ones[:, 0:k],
                                     start=False, stop=(t == 0))
                    nc.tensor.matmul(cbank[:, 0:k], bc[:],
                                     ones[:, 0:k], start=False, stop=(t == 0))
                    # recurrent contribution via q, p  (p_0 == 0, so skip at t=1)
                    if t > 0:
                        if p16_prev is not None:
                            nc.tensor.matmul(gbank[:, 0:k], wghp[:, 0:H],
                                             p16_prev[:, 0:k], start=False,
                                             stop=False)
                            nc.tensor.matmul(gbank[:, BL: BL + k],
                                             wghp[:, H: 2 * H],
                                             p16_prev[:, 0:k], start=False,
                                             stop=False)
                        nc.tensor.matmul(gbank[:, 0:k], wghq[:, 0:H],
                                         q16_prev[:, 0:k], start=False,
                                         stop=False)
                        nc.tensor.matmul(gbank[:, BL: BL + k], wghq[:, H: 2 * H],
                                         q16_prev[:, 0:k], start=False,
                                         stop=True)

                    r = rp.tile([H, BL], f32)
                    nc.scalar.activation(r[:, 0:k], gbank[:, 0:k],
                                         mybir.ActivationFunctionType.Sigmoid)
                    v = vp.tile([H, BL], f32)
                    nc.scalar.activation(v[:, 0:k], gbank[:, BL: BL + k],
                                         mybir.ActivationFunctionType.Sigmoid)

                    if t > 0:
                        if j > 0:
                            h_prev = yw[:, hs - BL: hs]
                        else:
                            h_prev = yw_prev[:, (CH - 1) * BL: CH * BL]
                        rh = rhp.tile([H, BL], f16)
                        nc.vector.tensor_mul(rh[:, 0:k], r[:, 0:k],
                                             h_prev[:, 0:k])
                        nc.tensor.matmul(cbank[:, 0:k], wch[:],
                                         rh[:, 0:k], start=False, stop=True)

                    c = cp.tile([H, BL], f32)
                    nc.scalar.activation(c[:, 0:k], cbank[:, 0:k],
                                         mybir.ActivationFunctionType.Tanh)

                    q16 = q16p.tile([H, BL], f16)
                    nc.vector.tensor_mul(q16[:, 0:k], v[:, 0:k], c[:, 0:k])

                    import concourse.mybir as _mb
                    if t > 0:
                        p32 = p32p.tile([H, BL], f32)
                        nc.vector.scalar_tensor_tensor(
                            p32[:, 0:k], v[:, 0:k], 1.0, h_prev[:, 0:k],
                            _mb.AluOpType.subtract, _mb.AluOpType.mult)
                        p16 = p16p.tile([H, BL], f16)
                        nc.vector.scalar_tensor_tensor(
                            p16[:, 0:k], v[:, 0:k], 1.0, h_prev[:, 0:k],
                            _mb.AluOpType.subtract, _mb.AluOpType.mult)
                        q32 = q32p.tile([H, BL], f32)
                        nc.gpsimd.tensor_mul(q32[:, 0:k], v[:, 0:k], c[:, 0:k])
                        nc.vector.tensor_sub(yw[:, hs: hs + k], q32[:, 0:k],
                                             p32[:, 0:k])
                    else:
                        p16 = None
                        nc.gpsimd.tensor_mul(yw[:, hs: hs + k], v[:, 0:k],
                                             c[:, 0:k])
                    q16_prev, p16_prev = q16, p16

                    # mask multiply per group
                    if j + 1 - group_start == MASK_GROUP or j == nsteps - 1:
                        g0 = group_start
                        gn = j + 1 - g0
                        tg = t0c + g0
                        mb = mbp.tile([H, MASK_GROUP * BL], f32)
                        nc.tensor.matmul(
                            mb[:, 0: gn * BL], ones[:],
                            mrow[:, tg * BL: (tg + gn) * BL],
                            start=True, stop=True)
                        nc.vector.tensor_mul(
                            yw[:, g0 * BL: (g0 + gn) * BL],
                            yw[:, g0 * BL: (g0 + gn) * BL],
                            mb[:, 0: gn * BL])
                        group_start = j + 1

                # store chunk
                half = (nslots * BL) // 2
                nc.scalar.dma_start(yT_d[:, t0c * BL: t0c * BL + half],
                                    yw[:, :half])
                nc.gpsimd.dma_start(
                    yT_d[:, t0c * BL + half: (t0c + nslots) * BL],
                    yw[:, half: nslots * BL])
                yw_prev = yw

    nc.compile()
    return nc


def _prepare(inputs):
    item_his_eb = np.asarray(inputs["item_his_eb"], dtype=np.float32)
    seq_len = np.asarray(inputs["seq_len"], dtype=np.int32)
    W_gate = np.asarray(inputs["W_gate"], dtype=np.float32)
    b_gate = np.asarray(inputs["b_gate"], dtype=np.float32)
    W_cand = np.asarray(inputs["W_cand"], dtype=np.float32)
    b_cand = np.asarray(inputs["b_cand"], dtype=np.float32)

    order = np.argsort(-seq_len, kind="stable")
    perms = [order[c::N_CORES] for c in range(N_CORES)]

    # common alive-prefix sizes
    k_common = np.zeros(T, dtype=np.int64)
    for c in range(N_CORES):
        Lc = seq_len[perms[c]]
        kc = (Lc[None, :] > np.arange(T)[:, None]).sum(axis=1)
        k_common = np.maximum(k_common, kc)
    k_common = np.minimum(_round_up(k_common, KROUND), BL)
    t_eff = int(seq_len.max())  # steps 0..t_eff-1 need compute

    # weight transforms (channels-on-partitions; u column block negated)
    wgx = W_gate[0:D, :].copy()
    wgh = W_gate[D: D + H, :].copy()
    wgx[:, H:] = -wgx[:, H:]
    wghq = wgh.copy()
    wghq[:, H:] = -wghq[:, H:]
    wghp = -wgh
    wghp[:, H:] = -wghp[:, H:]  # = [-Wh_r | +Wh_u]
    br = b_gate[0:H]
    bu = -b_gate[H: 2 * H]
    wcx = W_cand[0:D, :]
    wch = W_cand[D: D + H, :]
    bc = b_cand

    common = {
        "wgx": wgx.astype(np.float16), "wghq": wghq.astype(np.float16),
        "wghp": wghp.astype(np.float16), "wcx": wcx.astype(np.float16),
        "wch": wch.astype(np.float16),
        "br": br.reshape(1, H).astype(np.float16),
        "bu": bu.reshape(1, H).astype(np.float16),
        "bc": bc.reshape(1, H).astype(np.float16),
    }

    in_maps = []
    for c in range(N_CORES):
        p = perms[c]
        xc = item_his_eb[p]                      # [BL, T, D]
        xT = np.ascontiguousarray(xc.transpose(2, 1, 0)).reshape(D, T * BL)
        Lc = seq_len[p]
        mask = (np.arange(T)[:, None] < Lc[None, :])  # [T, BL]
        in_maps.append({
            "xT16": xT.astype(np.float16),
            "maskrow": mask.reshape(1, T * BL).astype(np.float16),
            **common,
        })
    return in_maps, perms, tuple(int(x) for x in k_common), t_eff


def kernel(**inputs) -> np.ndarray:
    from concourse.bass_utils import run_bass_kernel_spmd

    in_maps, perms, k_common, t_eff = _prepare(inputs)

    key = (k_common, t_eff)
    nc = _compiled_cache.get(key)
    if nc is None:
        nc = _build_program(list(k_common), t_eff)
        _compiled_cache[key] = nc

    res = run_bass_kernel_spmd(nc, in_maps, core_ids=list(range(N_CORES)))

    out = np.empty((B, T, H), dtype=np.float32)
    for c in range(N_CORES):
        yT = res.results[c]["yT"]                       # [H, T*BL]
        yc = yT.reshape(H, T, BL).transpose(2, 1, 0)    # [BL, T, H]
        out[perms[c]] = yc
    return out
